# revision 9
# baseline (speedup 1.0000x reference)
"""Trainium2 Bass kernel for BasicBlock(1w4a): quant-act conv3x3 + BN + ReLU.

Data-parallel over 8 NeuronCores (batch 32 -> 8 x 4). Each core packs 2
samples onto the 128 SBUF partitions (64 channels each) and runs the 3x3
conv as shifted matmuls with block-diagonal weights accumulating in PSUM.

Exactness: activations quantize to integers 0..15, weights binarize to +-1.
Both are exact in fp8e4m3, and PSUM accumulates in fp32 (sums bounded well
below 2^24), so the conv is bit-exact. The DoReFa scale (alpha/15) and
BatchNorm fold into a per-channel affine applied by ScalarE as
relu(scale*psum + bias). Output is stored fp16 (rel err ~2^-11, far under
the 2e-2 gate) and upcast to fp32 on host, halving output HBM traffic.

Spatial layout: each sample-pair is quantized (in 16-row slices, pipelined
against the matmuls of earlier slices) onto a per-pair 114x120 zero-padded
frame grid (112 data cols + 8 zero pad cols per row, plus top/bottom zero
halo rows), so a conv tap (dh, dw) is a single flat offset dh*120+dw into
the grid and all image-edge reads land in zeros.

HBM/issue efficiency: x is zero-padded on host to 120-wide rows
(layout-only transform) and DMA'd with a handful of large up-front
transfers into two whole-pair SBUF tiles -- descriptor generation
(~0.65us per dma_start on the issuing sequencer) stops gating the
pipeline, every descriptor is a multi-KB contiguous run, and the pad
columns flow through the quantize math as exact zeros.  Output DMAs ride
the sync ring (idle after the prefetch) so ScalarE's sequencer only runs
the PSUM->fp16 relu-affine drains.

Matmuls per 4-row output chunk (fp8 DoubleRow contracts 2 taps at once):
  3x DoubleRow pairs {(-1,dw),(+1,dw)}  rhs middle-dim step 240 (2 rows)
  1x DoubleRow pair  {(0,-1),(0,+1)}    via a +2-shifted copy of the grid
  1x normal          {(0,0)}
"""

import os

import numpy as np
import ml_dtypes

import concourse.bass as bass
import concourse.mybir as mybir
import concourse.tile as tile
from concourse import bacc
from concourse.bass_utils import run_bass_kernel_spmd

# ---- problem constants (hardcoded per harness contract) ----
N_CORES = 8
B_FULL = 32
B_SHARD = B_FULL // N_CORES  # 4
C = 64
H = 112
W = 112
BN_EPS = 1e-5

P = 128           # SBUF partitions
GW = 120          # padded grid row width (112 data + 8 zero pad)
SLICE = 16        # x-rows quantized per slice / output rows per mm block
NSL = H // SLICE  # 7 slices (= mm blocks) per sample-pair
NMM = 4 * GW      # matmul free dim per chunk (480)
HDR = 16          # zero header elems (catches tap reads at flat index -1)
TRL = 32          # zero trailer elems (catches tap reads past the grid)
FROWS = H + 2     # frame grid rows: 112 data + top/bottom zero halo
FGRID = FROWS * GW                 # 13680
FC2 = HDR + FGRID + TRL            # copy2 region start; delta 13712 %16==0
FSTORE = FC2 + FGRID

MAGIC = 12582912.0  # 1.5 * 2^23: x+MAGIC-MAGIC rounds to int, half-to-even

# staged prefetch row-chunks: fine-grained first (so quantize starts the
# moment the first rows land), coarse later (fewer issue slots)
PREFETCH = {
    0: [(0, 16), (16, 32), (32, 64), (64, 112)],
    1: [(0, 56), (56, 112)],
}

VARIANT = os.environ.get("KERNEL_VARIANT", "fp8dr")

_cache = {}


def _build_nc(variant):
    assert variant == "fp8dr"
    qdt = mybir.dt.float8e4

    nc = bacc.Bacc(None, target_bir_lowering=False)
    # x uploaded host-padded to 120-wide rows (pads zero)
    x = nc.dram_tensor("x", [B_SHARD, C, H, GW], mybir.dt.float32,
                       kind="ExternalInput")
    # all fp8 weights in one upload: 4 DoubleRow pair sets + 1 single (0,0)
    wdm_d = nc.dram_tensor("wdm", [P, 9 * P], mybir.dt.float8e4,
                           kind="ExternalInput")
    # [scale | bias] per-channel affine
    sb_d = nc.dram_tensor("sb", [P, 2], mybir.dt.float32,
                          kind="ExternalInput")
    y = nc.dram_tensor("y", [B_SHARD, C, H, W], mybir.dt.float16,
                       kind="ExternalOutput")

    with tile.TileContext(nc) as tc:
        with (
            tc.tile_pool(name="singles", bufs=1) as singles,
            tc.tile_pool(name="raws", bufs=2) as raw_pool,
            tc.tile_pool(name="qgs", bufs=2) as qg_pool,
            tc.tile_pool(name="outs", bufs=4) as out_pool,
            tc.tile_pool(name="psums", bufs=8, space="PSUM") as psum_pool,
        ):
            # ---- up-front issue of every input DMA (descriptor gen on
            # the sync sequencer is ~0.65us per dma_start; nothing else
            # may queue ahead of the prefetch) ----
            raws = {}
            for pr in range(B_SHARD // 2):
                raws[pr] = raw_pool.tile([P, H, GW], mybir.dt.float32,
                                         name=f"raw{pr}", tag="raw")
            x0 = x[0:2].rearrange("s c h w -> (s c) h w")
            nc.sync.dma_start(out=raws[0][:, 0:16, :], in_=x0[:, 0:16, :])
            wdm_t = singles.tile([P, 9 * P], mybir.dt.float8e4)
            nc.sync.dma_start(out=wdm_t[:, :], in_=wdm_d[:, :])
            sb_t = singles.tile([P, 2], mybir.dt.float32)
            nc.sync.dma_start(out=sb_t[:, :], in_=sb_d[:, :])
            for pr in range(B_SHARD // 2):
                x2 = x[2 * pr:2 * pr + 2].rearrange("s c h w -> (s c) h w")
                for (ra, rb) in PREFETCH[pr]:
                    if pr == 0 and ra == 0:
                        continue  # already issued above
                    nc.sync.dma_start(out=raws[pr][:, ra:rb, :],
                                      in_=x2[:, ra:rb, :])
            scale_t = sb_t[:, 0:1]
            bias_t = sb_t[:, 1:2]

            def emit_frame(pair):
                # per-pair fp8 quant frame: [hdr][114x120 grid][trl][copy2]
                qg = qg_pool.tile([P, FSTORE], qdt,
                                  name=f"qg{pair}", tag="qg")
                nc.gpsimd.memset(qg[:, 0:HDR], 0.0)
                nc.gpsimd.memset(qg[:, HDR + FGRID:FC2], 0.0)
                # top/bottom zero-halo rows of the grid and their images in
                # the +2-shifted copy
                nc.gpsimd.memset(qg[:, HDR:HDR + GW], 0.0)
                nc.gpsimd.memset(
                    qg[:, HDR + (FROWS - 1) * GW:HDR + FROWS * GW], 0.0)
                nc.gpsimd.memset(qg[:, FC2 - 2:FC2 - 2 + GW], 0.0)
                nc.gpsimd.memset(
                    qg[:, FC2 - 2 + (FROWS - 1) * GW:FC2 - 2 + FROWS * GW],
                    0.0)
                return qg

            def emit_quant(pair, sl, qg):
                # quantize x rows [16*sl, 16*sl+16) into frame grid rows
                # [16*sl+1, 16*sl+17)
                r0 = SLICE * sl
                raw = raws[pair]
                g0 = HDR + (r0 + 1) * GW       # frame position of this slice
                SR = SLICE // 2
                # t = max(15*x, 0): halves split ACT/DVE to balance load
                h0 = raw[:, r0:r0 + SR, :].rearrange("p a b -> p (a b)")
                h1 = raw[:, r0 + SR:r0 + SLICE, :] \
                    .rearrange("p a b -> p (a b)")
                nc.scalar.activation(
                    out=h0, in_=h0,
                    func=mybir.ActivationFunctionType.Relu,
                    scale=15.0,
                )
                nc.vector.tensor_scalar(
                    out=h1, in0=h1,
                    scalar1=15.0, scalar2=0.0,
                    op0=mybir.AluOpType.mult,
                    op1=mybir.AluOpType.max,
                )
                # full-slice: t = min(t,15) + MAGIC (fp32 add rounds, RNE)
                full = raw[:, r0:r0 + SLICE, :].rearrange("p a b -> p (a b)")
                nc.vector.tensor_scalar(
                    out=full, in0=full,
                    scalar1=15.0, scalar2=MAGIC,
                    op0=mybir.AluOpType.min, op1=mybir.AluOpType.add,
                )
                # q = t - MAGIC -> integers 0..15, exact in fp8; main grid
                # copy and +2-shifted copy (for the {(0,-1),(0,+1)}
                # DoubleRow pair).  Pads are zero in DRAM -> exact 0 here.
                nc.vector.tensor_scalar(
                    out=qg[:, g0:g0 + SLICE * GW], in0=full,
                    scalar1=MAGIC, scalar2=None,
                    op0=mybir.AluOpType.subtract,
                )
                nc.vector.tensor_scalar(
                    out=qg[:, g0 + (FC2 - 2 - HDR):
                           g0 + (FC2 - 2 - HDR) + SLICE * GW],
                    in0=full,
                    scalar1=MAGIC, scalar2=None,
                    op0=mybir.AluOpType.subtract,
                )

            def emit_mm(pair, blk, qg, last=False):
                y2 = y[2 * pair:2 * pair + 2].rearrange(
                    "s c h w -> (s c) h w")
                r0 = SLICE * blk
                ot = out_pool.tile([P, SLICE, W], mybir.dt.float16,
                                   name=f"ot{pair}_{blk}", tag="ot")
                for c in range(SLICE // 4):
                    ch = (r0 // 4) + c           # frame-global 4-row chunk
                    ps = psum_pool.tile([P, NMM], mybir.dt.float32,
                                        name=f"ps{pair}_{blk}_{c}",
                                        tag="ps")
                    # 3 DoubleRow pair-matmuls: taps (-1,dw)+(+1,dw)
                    for i, dw in enumerate((-1, 0, 1)):
                        base = HDR + (4 * ch) * GW + dw
                        rhs = qg[:, base:base + NMM]
                        v = rhs.ap
                        v[1] = [2 * GW, 2]
                        v.append([1, NMM])
                        rhs.ap = v
                        lhsT = wdm_t[:, i * 2 * P:(i + 1) * 2 * P] \
                            .rearrange("p (a b) -> p a b", a=2)
                        nc.tensor.matmul(
                            ps[:, :], lhsT=lhsT, rhs=rhs,
                            start=(i == 0), stop=False,
                            perf_mode=mybir.MatmulPerfMode.DoubleRow,
                        )
                    # DoubleRow pair: taps (0,-1)+(0,+1) via copy2
                    base = HDR + (4 * ch + 1) * GW - 1
                    rhs = qg[:, base:base + NMM]
                    v = rhs.ap
                    v[1] = [FC2 - HDR, 2]
                    v.append([1, NMM])
                    rhs.ap = v
                    lhsT = wdm_t[:, 3 * 2 * P:4 * 2 * P] \
                        .rearrange("p (a b) -> p a b", a=2)
                    nc.tensor.matmul(
                        ps[:, :], lhsT=lhsT, rhs=rhs,
                        start=False, stop=False,
                        perf_mode=mybir.MatmulPerfMode.DoubleRow,
                    )
                    # normal matmul: tap (0,0)
                    base = HDR + (4 * ch + 1) * GW
                    nc.tensor.matmul(
                        ps[:, :], lhsT=wdm_t[:, 8 * P:9 * P],
                        rhs=qg[:, base:base + NMM],
                        start=False, stop=True,
                    )
                    pv = ps.rearrange("p (r c) -> p r c", c=GW)
                    nc.scalar.activation(
                        out=ot[:, 4 * c:4 * c + 4, :],
                        in_=pv[:, :, 0:W],
                        func=mybir.ActivationFunctionType.Relu,
                        bias=bias_t,
                        scale=scale_t,
                    )
                # output DMAs ride the sync ring: its sequencer is idle
                # after the up-front prefetch, keeping ScalarE free
                if last:
                    # drain the final block chunk-by-chunk so the last
                    # transfer overlaps the remaining chunks' ACTs
                    for c in range(SLICE // 4):
                        nc.sync.dma_start(
                            out=y2[:, r0 + 4 * c:r0 + 4 * c + 4, :],
                            in_=ot[:, 4 * c:4 * c + 4, :],
                        )
                else:
                    nc.sync.dma_start(
                        out=y2[:, r0:r0 + SLICE, :],
                        in_=ot[:, :, :],
                    )

            # Software-pipelined emission. mm block k reads frame grid rows
            # [16k, 16k+17], i.e. slices k-1, k and the first row of slice
            # k+1 -- so mm(k) is emitted after quant(k+1).
            frames = {pr: None for pr in range(B_SHARD // 2)}
            work = [(pr, sl) for pr in range(B_SHARD // 2)
                    for sl in range(NSL)]
            pending = []
            for pr, sl in work:
                if sl == 0:
                    frames[pr] = emit_frame(pr)
                emit_quant(pr, sl, frames[pr])
                if sl >= 1:
                    pending.append((pr, sl - 1, frames[pr]))
                if sl == NSL - 1:
                    pending.append((pr, sl, frames[pr]))
                while len(pending) > 1:
                    emit_mm(*pending.pop(0))
            for pr_, blk_, fr_ in pending:
                emit_mm(pr_, blk_, fr_,
                        last=(pr_ == B_SHARD // 2 - 1 and blk_ == NSL - 1))

    nc.finalize()
    return nc


def _blockdiag(blk64):
    out = np.zeros((P, P), dtype=np.float32)
    out[0:64, 0:64] = blk64
    out[64:128, 64:128] = blk64
    return out


def _host_prep(w, gamma, beta, bn_mean, bn_var):
    w = np.asarray(w, dtype=np.float32)
    alpha = np.float32(np.mean(np.abs(w)))
    ws = np.sign(w).astype(np.float32)           # [co, ci, 3, 3]
    inv = (np.asarray(gamma, np.float32)
           / np.sqrt(np.asarray(bn_var, np.float32) + np.float32(BN_EPS)))
    scale_c = (inv * (alpha / np.float32(15.0))).astype(np.float32)
    bias_c = (np.asarray(beta, np.float32)
              - np.asarray(bn_mean, np.float32) * inv).astype(np.float32)
    sb = np.zeros((P, 2), dtype=np.float32)
    sb[:, 0] = np.concatenate([scale_c, scale_c])
    sb[:, 1] = np.concatenate([bias_c, bias_c])

    # pair sets: [(dh=-1,dw),(dh=+1,dw)] for dw in 0..2, then
    # [(0,-1),(0,+1)]; single = (0,0).  w index [co, ci, dh+1, dw+1].
    wdm = np.zeros((P, 9, P), dtype=np.float32)
    for i, dw in enumerate(range(3)):
        wdm[:, 2 * i + 0, :] = _blockdiag(ws[:, :, 0, dw].T)  # dh=-1
        wdm[:, 2 * i + 1, :] = _blockdiag(ws[:, :, 2, dw].T)  # dh=+1
    wdm[:, 6, :] = _blockdiag(ws[:, :, 1, 0].T)               # (0,-1)
    wdm[:, 7, :] = _blockdiag(ws[:, :, 1, 2].T)               # (0,+1)
    wdm[:, 8, :] = _blockdiag(ws[:, :, 1, 1].T)               # (0,0)

    wdm8 = np.ascontiguousarray(
        wdm.reshape(P, 9 * P).astype(ml_dtypes.float8_e4m3))
    return wdm8, sb


_last_results = None  # test harness peeks at this for profile data


def kernel(x, w, gamma, beta, bn_mean, bn_var):
    global _last_results
    variant = VARIANT
    if variant not in _cache:
        _cache[variant] = _build_nc(variant)
    nc = _cache[variant]

    wdm8, sb = _host_prep(w, gamma, beta, bn_mean, bn_var)
    x = np.asarray(x, dtype=np.float32)
    # layout-only host transform: zero-pad rows 112 -> 120 so slice DMAs
    # are single contiguous runs and grid pads arrive as exact zeros
    xp = np.zeros((B_FULL, C, H, GW), dtype=np.float32)
    xp[:, :, :, :W] = x

    in_maps = []
    for i in range(N_CORES):
        m = {
            "x": np.ascontiguousarray(xp[i * B_SHARD:(i + 1) * B_SHARD]),
            "wdm": wdm8,
            "sb": sb,
        }
        in_maps.append(m)
    res = run_bass_kernel_spmd(nc, in_maps, core_ids=list(range(N_CORES)))
    _last_results = res
    return np.concatenate(
        [res.results[i]["y"] for i in range(N_CORES)],
        axis=0).astype(np.float32)
